# revision 50
# baseline (speedup 1.0000x reference)
"""GCMCGraphConv Bass kernel for 8 TRN2 NeuronCores.

Computes: h = ci * segment_sum((weight * cj)[src], dst)  for a random
graph with N=100000 nodes, F=128 features, E=1600000 edges.

Strategy (v11 — paired direct gather):
  - host precomputes wc = bf16(weight * cj); the device gathers edge
    rows straight from a per-core staged copy (no conversion phase)
  - core c owns dst rows [c*12500, (c+1)*12500); edges partitioned by
    dst owner; per-core dst->block packing keeps all but one overflow
    block at <= 2048 edges
  - the gpsimd dma_gather ucode costs ~1.6ns per index serialized on
    the one GpSimd engine, which makes descriptor COUNT the kernel's
    bottleneck.  So each 512B descriptor (elem_size=256, elem_step=128
    overlapping rows) fetches TWO consecutive rows of a host-chosen
    ordering B_w, and the host pairs up edges of the same (block,
    window) bin so both halves are real edges.  Pairing is a greedy
    matching under a linear-forest constraint per (core, window):
    every row has at most 2 neighbors in B_w and no cycles.
  - gather indices are int16, so rows live in one of 4 windows by src
    value (25600-stride, 32768-wide overlap); edges in overlap zones
    can be assigned to either window, which the host uses to fill
    windows 0-2 of every block to exactly 2 pair-columns (512 B-slots)
    and leave the remainder to window 3
  - per block one fused is_equal builds the one-hot (DVE), w_b bf16
    matmuls accumulate the segment sum in PSUM, the scalar engine
    applies ci, then the output DMA writes the block; the host
    un-permutes rows of the returned h
"""

import os
import sys

import numpy as np

sys.path.insert(0, "/opt/trn_rl_repo")

from concourse import bacc, bass, mybir  # noqa: E402
import concourse.tile as tile  # noqa: E402
from concourse.bass_utils import run_bass_kernel_spmd  # noqa: E402

N_NODES = 100000
FEAT = 128
N_CORES = 8
DST_PER_CORE = N_NODES // N_CORES  # 12500
P = 128
N_BLOCKS = (DST_PER_CORE + P - 1) // P  # 98
DST_PAD = N_BLOCKS * P  # 12544

SEG = 4
WIN = 32768  # int16-addressable gather window
BASES = [0, 18432, 44032, 69632]  # window start rows (overlapping)
WSIZES = [min(b + WIN, N_NODES) - b for b in BASES]
WSTARTS = np.concatenate([[0], np.cumsum(WSIZES)]).astype(np.int64)
NSTAGE = int(WSTARTS[-1]) + 1  # +1 pad row for the last pair descriptor
PIECE = int(os.environ.get("KERNEL_PIECE", "8"))  # pair-columns per
# dma_gather (1024 idx = the gather ucode idx ring limit); at the
# current DMA load the drain no longer blocks the descriptor ring

LAST_EXEC_NS = None


def _ensure_ntff_hook():
    """Shim antenv.axon_hooks if the image's antenv predates it."""
    import types

    try:
        from antenv.axon_hooks import get_axon_ntff_profile_hook  # noqa: F401

        return
    except ImportError:
        pass
    try:
        import antenv

        mod = types.ModuleType("antenv.axon_hooks")
        _hook = [None]
        mod.set_axon_ntff_profile_hook = lambda h: _hook.__setitem__(0, h)
        mod.get_axon_ntff_profile_hook = lambda: _hook[0]
        antenv.axon_hooks = mod
        sys.modules["antenv.axon_hooks"] = mod
        from trn_agent_boot.trn_boot import _ntff_profile_via_ctypes

        mod.set_axon_ntff_profile_hook(
            _ntff_profile_via_ctypes("/opt/axon/libaxon_pjrt.so")
        )
    except Exception:
        pass


def _build_program(sched) -> bass.Bass:
    """One SPMD program; every core runs it on its own edge shard."""
    nc = bacc.Bacc(num_swdge_queues=4)
    f32 = mybir.dt.float32
    bf16 = mybir.dt.bfloat16
    i16 = mybir.dt.int16

    caps_p = sched["caps_p"]  # [N_BLOCKS, SEG] pair-cols per (block, window)
    acts = sched["acts"]  # per block: list of active (window, paircol, half)
    w_b = np.asarray([len(a) for a in acts])  # matmuls per block
    maxw = int(w_b.max())
    col_off = np.concatenate([[0], np.cumsum(w_b)])
    ncols = int(col_off[-1])
    # pair-col index of (b, s, 0) within window s's gather stream
    prefix_p = sched["prefix_p"]
    border = sched["border"]
    n_pcols = prefix_p[N_BLOCKS]  # [SEG]
    n_pieces = [(int(nq) + PIECE - 1) // PIECE for nq in n_pcols]
    ipp = PIECE * P // 16  # idx cols per piece (64)
    idx_off = np.concatenate([[0], np.cumsum([nq * ipp for nq in n_pieces])])
    idxcols = int(idx_off[-1])

    w_d = nc.declare_dram_parameter("w", [NSTAGE, FEAT], bf16, isOutput=False)
    gidx_d = nc.declare_dram_parameter("gidx", [P, idxcols], i16, isOutput=False)
    dstloc_d = nc.declare_dram_parameter("dstloc", [P, ncols], bf16, isOutput=False)
    cib_d = nc.declare_dram_parameter("cib", [P, N_BLOCKS], f32, isOutput=False)
    iota_d = nc.declare_dram_parameter("iota", [P, maxw * P], bf16, isOutput=False)
    h_d = nc.declare_dram_parameter("h", [DST_PAD, FEAT], f32, isOutput=True)

    with tile.TileContext(nc) as tc:
        with (
            tc.tile_pool(name="meta", bufs=1) as meta,
            tc.tile_pool(name="gather", bufs=6) as gpool,
            tc.tile_pool(name="work", bufs=3) as work,
            tc.tile_pool(name="out", bufs=3) as opool,
            tc.tile_pool(name="psum", bufs=4, space="PSUM") as psum,
        ):
            gidx = meta.tile([P, idxcols], i16)
            dstloc = meta.tile([P, ncols], bf16)
            cib = meta.tile([P, N_BLOCKS], f32)
            # head pieces first so the first gathers start early
            for s in range(SEG):
                lo = int(idx_off[s])
                mid = min(lo + 2 * ipp, int(idx_off[s + 1]))
                nc.sync.dma_start(out=gidx[:, lo:mid], in_=gidx_d[:, lo:mid])
            nc.sync.dma_start(out=dstloc[:], in_=dstloc_d[:])
            for s in range(SEG):
                mid = min(int(idx_off[s]) + 2 * ipp, int(idx_off[s + 1]))
                hi = int(idx_off[s + 1])
                if hi > mid:
                    nc.sync.dma_start(out=gidx[:, mid:hi], in_=gidx_d[:, mid:hi])
            nc.sync.dma_start(out=cib[:], in_=cib_d[:])

            # iota[p, c*128 + j] = j  (dst slot within block), host-built
            iota = meta.tile([P, maxw * P], bf16)
            nc.sync.dma_start(out=iota[:], in_=iota_d[:])

            # issue all paired gathers; Tile paces them via pool bufs.
            # One 512B descriptor per pair-slot: rows B[t], B[t+1].
            gts: list[dict] = [{} for _ in range(SEG)]
            for pc in range(max(n_pieces)):
                for s in range(SEG):
                    if pc >= n_pieces[s]:
                        continue
                    npair = min(PIECE, int(n_pcols[s]) - pc * PIECE)
                    gt = gpool.tile([P, PIECE * 2 * FEAT], bf16, tag=f"gw{s}")
                    in_ap = bass.AP(
                        w_d[:, :].tensor,
                        int(WSTARTS[s]) * FEAT,
                        [(FEAT, WSIZES[s]), (1, 2 * FEAT)],
                    )
                    co = int(idx_off[s]) + pc * ipp
                    nc.gpsimd.dma_gather(
                        gt[:, : npair * 2 * FEAT].rearrange(
                            "p (m f) -> p m f", f=2 * FEAT
                        ),
                        in_ap,
                        gidx[:, co : co + npair * P // 16],
                        npair * P,
                        npair * P,
                        2 * FEAT,
                        elem_step=FEAT,
                        queue_num=s,
                    )
                    gts[s][pc] = gt

            for b in border:
                wb = int(w_b[b])
                co = int(col_off[b])
                onehot = work.tile([P, maxw * P], bf16, tag="onehot")
                nc.vector.tensor_tensor(
                    out=onehot[:, : wb * P].rearrange("p (m f) -> p m f", f=P),
                    in0=dstloc[:, co : co + wb].to_broadcast([P, wb, P]),
                    in1=iota[:, : wb * P].rearrange("p (m f) -> p m f", f=P),
                    op=mybir.AluOpType.is_equal,
                )
                acc = psum.tile([P, FEAT], f32, tag="acc")
                for j, (s, jj, half) in enumerate(acts[b]):
                    jp = int(prefix_p[b, s]) + jj  # global pair-col
                    gt = gts[s][jp // PIECE]
                    off = jp % PIECE
                    nc.tensor.matmul(
                        out=acc[:],
                        lhsT=onehot[:, j * P : (j + 1) * P],
                        rhs=gt[
                            :,
                            off * 2 * FEAT + half * FEAT : off * 2 * FEAT
                            + (half + 1) * FEAT,
                        ],
                        start=(j == 0),
                        stop=(j == wb - 1),
                    )
                ho = opool.tile([P, FEAT], f32, tag="ho")
                nc.scalar.mul(ho[:], acc[:], cib[:, b : b + 1])
                nc.sync.dma_start(out=h_d[b * P : (b + 1) * P, :], in_=ho[:])
    return nc


class _DSU:
    __slots__ = ("p",)

    def __init__(self, n):
        self.p = list(range(n))

    def find(self, x):
        p = self.p
        while p[x] != x:
            p[x] = p[p[x]]
            x = p[x]
        return x

    def union(self, a, b):
        self.p[self.find(a)] = self.find(b)


def _prep_inputs(weight, cj, ci, src, dst):
    """Partition edges by dst owner; pair edges; build metadata."""
    import ml_dtypes

    order = np.argsort(dst, kind="stable")
    ds = dst[order].astype(np.int64)
    ss = src[order].astype(np.int64)
    core_bounds = np.searchsorted(ds, np.arange(N_CORES + 1) * DST_PER_CORE)

    percore = []
    perms = []
    for c in range(N_CORES):
        a, b = core_bounds[c], core_bounds[c + 1]
        d_local = ds[a:b] - c * DST_PER_CORE
        g = ss[a:b]

        # dst->block packing: heaviest 128 dsts to the overflow block,
        # snake the rest so all other blocks carry <= 2048 edges.
        deg = np.bincount(d_local, minlength=DST_PER_CORE)
        order_d = np.argsort(-deg, kind="stable")
        blk_of = np.empty(DST_PER_CORE, dtype=np.int64)
        slot_of = np.empty(DST_PER_CORE, dtype=np.int64)
        hot = order_d[:P]
        blk_of[hot] = N_BLOCKS - 1
        slot_of[hot] = np.arange(P)
        rest = order_d[P:]
        nb = N_BLOCKS - 1
        for i in range(0, len(rest), nb):
            seg_d = rest[i : i + nb]
            row = i // nb
            blks = np.arange(len(seg_d))
            if row % 2:
                blks = nb - 1 - blks
            blk_of[seg_d] = blks
            slot_of[seg_d] = row
        perms.append((blk_of, slot_of))

        block = blk_of[d_local]
        o2 = np.lexsort((g, block))
        d_local, g, block = d_local[o2], g[o2], block[o2]
        bb = np.searchsorted(block, np.arange(N_BLOCKS + 1))
        percore.append((d_local, g, bb))

    # --- pairing + window fill -------------------------------------------
    # caps_p in pair-columns; windows 0-2 start at 2 and bump on overflow
    caps_p = np.full((N_BLOCKS, SEG), 2, dtype=np.int64)
    for attempt in range(6):
        overflow = np.zeros((N_BLOCKS, 3), dtype=bool)
        results = []  # per core: (pairs, halves) per (block, window)
        load3 = np.zeros((N_CORES, N_BLOCKS), dtype=np.int64)
        for c in range(N_CORES):
            d_local, g, bb = percore[c]
            # per-window pairing state over VIRTUAL row ids: originals
            # 0..WSIZE-1 plus up to BUD duplicated copies (a copy of a
            # row gets fresh degree-2 capacity in the B_w ordering)
            BUD = int(os.environ.get("KERNEL_BUD", "8000"))
            degv = [
                np.zeros(WSIZES[s] + BUD + 1, dtype=np.int8) for s in range(SEG)
            ]
            dsu = [_DSU(WSIZES[s] + BUD + 1) for s in range(SEG)]
            curv = [
                np.full(WSIZES[s], -1, dtype=np.int64) for s in range(SEG)
            ]
            copy_origs = [[] for _ in range(SEG)]
            placed = np.zeros(len(g), dtype=bool)
            core_res = [[None] * SEG for _ in range(N_BLOCKS)]
            for s in range(SEG):
                lo_v, hi_v = BASES[s], BASES[s] + WIN
                nxt = BASES[s + 1] if s < 3 else N_NODES
                norig = WSIZES[s]
                D = degv[s]
                U = dsu[s]
                cur = curv[s]
                cpo = copy_origs[s]
                for blk in range(N_BLOCKS):
                    i0, i1 = bb[blk], bb[blk + 1]
                    idxs = np.arange(i0, i1)[~placed[i0:i1]]
                    vals = g[idxs]
                    idxs = idxs[(vals >= lo_v) & (vals < hi_v)]
                    cap_slots = int(caps_p[blk, s]) * P
                    slots = []  # [e1, e2, vid1, vid2]; one descriptor each
                    open_h = []  # indices of slots missing a second half
                    for e in idxs:
                        u0 = int(g[e]) - lo_v
                        u = cur[u0] if cur[u0] >= 0 else u0
                        if D[u] >= 2 and len(cpo) < BUD:
                            u = norig + len(cpo)
                            cpo.append(u0)
                            cur[u0] = u
                        done = False
                        if D[u] < 2:
                            for t in range(len(open_h) - 1, -1, -1):
                                se = slots[open_h[t]]
                                v = se[2]
                                if D[v] >= 2:
                                    # re-key the stale half to a fresh copy
                                    if len(cpo) < BUD:
                                        v0 = v if v < norig else cpo[v - norig]
                                        v = norig + len(cpo)
                                        cpo.append(v0)
                                        cur[v0] = v
                                        se[2] = v
                                    else:
                                        open_h.pop(t)
                                        continue
                                if v == u or U.find(u) == U.find(v):
                                    if len(open_h) - t >= 16:
                                        break
                                    continue
                                se[1] = e
                                se[3] = u
                                D[u] += 1
                                D[v] += 1
                                U.union(u, v)
                                open_h.pop(t)
                                placed[e] = True
                                done = True
                                break
                        if done:
                            continue
                        if s == 3 or len(slots) < cap_slots:
                            open_h.append(len(slots))
                            slots.append([e, -1, u, -1])
                            placed[e] = True
                        elif int(g[e]) < nxt:
                            # a must-edge that neither fit nor paired
                            overflow[blk, s] = True
                        # else: eligible for the next window; leave it
                    pairs = [tuple(sl) for sl in slots if sl[1] >= 0]
                    halves = [(sl[0], sl[2]) for sl in slots if sl[1] < 0]
                    core_res[blk][s] = (pairs, halves)
                    if s == 3:
                        load3[c, blk] = len(slots)
            if not overflow.any():
                assert placed.all(), f"core {c}: {int((~placed).sum())} edges lost"
            results.append((core_res, copy_origs))
        if not overflow.any():
            break
        for blk in range(N_BLOCKS):
            for s in range(3):
                if overflow[blk, s]:
                    caps_p[blk, s] += 1
    caps_p[:, 3] = np.maximum(1, -(-load3.max(axis=0) // P))

    # active chunk columns: second halves of a pair-col carry edges only
    # where some core placed a pair there (union over cores keeps the
    # layout SPMD-uniform); dead columns get no one-hot and no matmul
    maxpb = np.zeros((N_BLOCKS, SEG), dtype=np.int64)
    maxsl = np.zeros((N_BLOCKS, SEG), dtype=np.int64)
    for c in range(N_CORES):
        for blk in range(N_BLOCKS):
            for s in range(SEG):
                pairs, halves = results[c][0][blk][s]
                maxpb[blk, s] = max(maxpb[blk, s], len(pairs))
                maxsl[blk, s] = max(maxsl[blk, s], len(pairs) + len(halves))
    acts = []
    colmap = {}
    col_off = [0]
    for blk in range(N_BLOCKS):
        al = []
        for s in range(SEG):
            for jj in range(int(caps_p[blk, s])):
                if maxsl[blk, s] > jj * P:
                    colmap[(blk, s, jj, 0)] = col_off[-1] + len(al)
                    al.append((s, jj, 0))
                if maxpb[blk, s] > jj * P:
                    colmap[(blk, s, jj, 1)] = col_off[-1] + len(al)
                    al.append((s, jj, 1))
        acts.append(al)
        col_off.append(col_off[-1] + len(al))
    col_off = np.asarray(col_off)
    ncols = int(col_off[-1])
    w_b = np.diff(col_off)
    n_pcols = caps_p.sum(axis=0)
    n_pieces = [(int(nq) + PIECE - 1) // PIECE for nq in n_pcols]
    ipp = PIECE * P // 16
    idx_off = np.concatenate([[0], np.cumsum([nq * ipp for nq in n_pieces])])
    idxcols = int(idx_off[-1])

    border = [N_BLOCKS - 1] + list(range(N_BLOCKS - 1))
    prefix_p = np.zeros((N_BLOCKS + 1, SEG), dtype=np.int64)
    run = np.zeros(SEG, dtype=np.int64)
    for blk in border:
        prefix_p[blk] = run
        run += caps_p[blk]
    prefix_p[N_BLOCKS] = run
    sched = {"caps_p": caps_p, "acts": acts, "prefix_p": prefix_p,
             "border": border}
    maxw = int(w_b.max())
    iota_arr = np.tile(np.arange(P, dtype=np.float32), (P, maxw)).astype(
        ml_dtypes.bfloat16
    )

    cj_flat = cj.reshape(-1).astype(np.float32)
    ci_flat = ci.reshape(-1).astype(np.float32)
    wc = (weight * cj_flat[:, None]).astype(ml_dtypes.bfloat16)

    in_maps = []
    npairs_tot = 0
    for c in range(N_CORES):
        blk_of, slot_of = perms[c]
        d_local, g, bb = percore[c]
        core_res, copy_origs = results[c]

        # B_w orderings from the pairing adjacencies (linear forest over
        # virtual row ids: originals then duplicated copies)
        posB = []
        stage_rows = np.empty(NSTAGE, dtype=np.int64)
        for s in range(SEG):
            nw = WSIZES[s]
            cpo = copy_origs[s]
            nv = nw + len(cpo)
            orig_of = np.concatenate(
                [np.arange(nw, dtype=np.int64), np.asarray(cpo, dtype=np.int64)]
            )
            A = {}
            refd = set()
            for blk in range(N_BLOCKS):
                pairs, halves = core_res[blk][s]
                for _, _, v1, v2 in pairs:
                    A.setdefault(v1, []).append(v2)
                    A.setdefault(v2, []).append(v1)
                    refd.add(v1)
                    refd.add(v2)
                for _, vv in halves:
                    refd.add(vv)
            pos = np.full(nv + 1, -1, dtype=np.int64)
            cur = 0
            visited = np.zeros(nv, dtype=bool)
            stage_win = np.zeros(nw, dtype=np.int64)  # default row 0 of window
            # path endpoints first (degree 1); cycles are prevented by the
            # DSU, so every component is a path
            for start in A:
                if visited[start] or len(A[start]) != 1:
                    continue
                node, prev = start, -1
                while True:
                    pos[node] = cur
                    stage_win[cur] = orig_of[node]
                    cur += 1
                    visited[node] = True
                    nxt_n = -1
                    for cand in A[node]:
                        if cand != prev and not visited[cand]:
                            nxt_n = cand
                            break
                    if nxt_n < 0:
                        break
                    prev, node = node, nxt_n
            for vv in refd:
                if pos[vv] < 0:
                    pos[vv] = cur
                    stage_win[cur] = orig_of[vv]
                    cur += 1
            assert cur <= nw, f"window {s}: {cur} > {nw} B positions"
            posB.append(pos)
            stage_rows[WSTARTS[s] : WSTARTS[s + 1]] = stage_win + BASES[s]
        stage_rows[-1] = 0
        wstage = wc[stage_rows]

        dstloc = np.full((P, ncols), -1, dtype=ml_dtypes.bfloat16)
        pairidx = np.zeros((P, int(n_pcols.sum())), dtype=np.int16)
        pcol_off = np.concatenate([[0], np.cumsum(n_pcols)])
        for s in range(SEG):
            pos = posB[s]
            qbase = int(pcol_off[s])
            for blk in range(N_BLOCKS):
                pairs, halves = core_res[blk][s]
                npairs_tot += len(pairs)
                q0 = qbase + int(prefix_p[blk, s])
                i = 0
                for e1, e2, v1, v2 in pairs:
                    u = pos[v1]
                    v = pos[v2]
                    jj, pp = i // P, i % P
                    if u + 1 == v:
                        t, h0, h1 = u, e1, e2
                    else:
                        assert v + 1 == u, (u, v)
                        t, h0, h1 = v, e2, e1
                    pairidx[pp, q0 + jj] = t
                    dstloc[pp, colmap[(blk, s, jj, 0)]] = slot_of[
                        d_local[h0]
                    ].astype(ml_dtypes.bfloat16)
                    dstloc[pp, colmap[(blk, s, jj, 1)]] = slot_of[
                        d_local[h1]
                    ].astype(ml_dtypes.bfloat16)
                    i += 1
                for e, vv in halves:
                    t = pos[vv]
                    jj, pp = i // P, i % P
                    pairidx[pp, q0 + jj] = t
                    dstloc[pp, colmap[(blk, s, jj, 0)]] = slot_of[
                        d_local[e]
                    ].astype(ml_dtypes.bfloat16)
                    i += 1

        # pack pair indices: per (window, piece), idx j at
        # [16*rep + j%16, j//16]; j = (paircol_within_piece*128 + p)
        gidx = np.zeros((P, idxcols), dtype=np.int16)
        for s in range(SEG):
            vals = pairidx[
                :, int(pcol_off[s]) : int(pcol_off[s + 1])
            ].T.reshape(-1)
            vals = np.pad(vals, (0, n_pieces[s] * PIECE * P - len(vals)))
            block16 = vals.reshape(n_pieces[s] * ipp, 16).T
            gidx[:, int(idx_off[s]) : int(idx_off[s + 1])] = np.tile(
                block16, (8, 1)
            )

        ci_core = ci_flat[c * DST_PER_CORE : (c + 1) * DST_PER_CORE]
        cib_arr = np.zeros((N_BLOCKS, P), dtype=np.float32)
        cib_arr[blk_of, slot_of] = ci_core
        cib = cib_arr.T.copy()

        in_maps.append(
            {
                "w": wstage,
                "gidx": gidx,
                "dstloc": dstloc,
                "cib": cib,
                "iota": iota_arr,
            }
        )
    if int(os.environ.get("KERNEL_STATS", "0")):
        tot_slots = int(n_pcols.sum()) * P * N_CORES
        print(
            f"[prep] pair-cols/core={int(n_pcols.sum())} slots/core="
            f"{int(n_pcols.sum()) * P} pairs(all cores)={npairs_tot} "
            f"edges={len(ds)} fill={(len(ds) + npairs_tot) / tot_slots:.3f}"
        )
    return in_maps, sched, perms


def _maybe_enable_ldw_opt():
    if not int(os.environ.get("KERNEL_LDW", "0")):
        return
    import concourse.bass_utils as _bu

    if getattr(_bu, "_ldw_patched", False):
        return
    _orig = _bu.run_command

    def _patched(argv, **kw):
        argv = [
            "--enable-ldw-opt=true" if a == "--enable-ldw-opt=false" else a
            for a in argv
        ]
        return _orig(argv, **kw)

    _bu.run_command = _patched
    _bu._ldw_patched = True


def kernel(weight, cj, ci, src, dst):
    global LAST_EXEC_NS
    _maybe_enable_ldw_opt()
    weight = np.asarray(weight, dtype=np.float32)
    cj = np.asarray(cj, dtype=np.float32)
    ci = np.asarray(ci, dtype=np.float32)
    src = np.asarray(src, dtype=np.int32)
    dst = np.asarray(dst, dtype=np.int32)

    in_maps, sched, perms = _prep_inputs(weight, cj, ci, src, dst)
    nc = _build_program(sched)
    nc.finalize()
    trace = bool(int(os.environ.get("KERNEL_TRACE", "0")))
    if trace:
        _ensure_ntff_hook()
    try:
        res = run_bass_kernel_spmd(
            nc, in_maps, core_ids=list(range(N_CORES)), trace=trace
        )
    except Exception:
        # one retry without tracing — transient device errors
        # (NRT_EXEC_UNIT_UNRECOVERABLE) usually clear on the next attempt
        res = run_bass_kernel_spmd(
            nc, in_maps, core_ids=list(range(N_CORES)), trace=False
        )
    LAST_EXEC_NS = res.exec_time_ns
    parts = []
    for c in range(N_CORES):
        blk_of, slot_of = perms[c]
        h = res.results[c]["h"]
        parts.append(h[blk_of * P + slot_of])
    return np.concatenate(parts, axis=0).astype(np.float32)


# revision 51
# speedup vs baseline: 1.1187x; 1.1187x over previous
"""GCMCGraphConv Bass kernel for 8 TRN2 NeuronCores.

Computes: h = ci * segment_sum((weight * cj)[src], dst)  for a random
graph with N=100000 nodes, F=128 features, E=1600000 edges.

Strategy (v11 — paired direct gather):
  - host precomputes wc = bf16(weight * cj); the device gathers edge
    rows straight from a per-core staged copy (no conversion phase)
  - core c owns dst rows [c*12500, (c+1)*12500); edges partitioned by
    dst owner; per-core dst->block packing keeps all but one overflow
    block at <= 2048 edges
  - the gpsimd dma_gather ucode costs ~1.6ns per index serialized on
    the one GpSimd engine, which makes descriptor COUNT the kernel's
    bottleneck.  So each 512B descriptor (elem_size=256, elem_step=128
    overlapping rows) fetches TWO consecutive rows of a host-chosen
    ordering B_w, and the host pairs up edges of the same (block,
    window) bin so both halves are real edges.  Pairing is a greedy
    matching under a linear-forest constraint per (core, window):
    every row has at most 2 neighbors in B_w and no cycles.
  - gather indices are int16, so rows live in one of 4 windows by src
    value (25600-stride, 32768-wide overlap); edges in overlap zones
    can be assigned to either window, which the host uses to fill
    windows 0-2 of every block to exactly 2 pair-columns (512 B-slots)
    and leave the remainder to window 3
  - per block one fused is_equal builds the one-hot (DVE), w_b bf16
    matmuls accumulate the segment sum in PSUM, the scalar engine
    applies ci, then the output DMA writes the block; the host
    un-permutes rows of the returned h
"""

import os
import sys

import numpy as np

sys.path.insert(0, "/opt/trn_rl_repo")

from concourse import bacc, bass, mybir  # noqa: E402
import concourse.tile as tile  # noqa: E402
from concourse.bass_utils import run_bass_kernel_spmd  # noqa: E402

N_NODES = 100000
FEAT = 128
N_CORES = 8
DST_PER_CORE = N_NODES // N_CORES  # 12500
P = 128
N_BLOCKS = (DST_PER_CORE + P - 1) // P  # 98
DST_PAD = N_BLOCKS * P  # 12544

SEG = 4
WIN = 32768  # int16-addressable gather window
BASES = [0, 18432, 44032, 69632]  # window start rows (overlapping)
WSIZES = [min(b + WIN, N_NODES) - b for b in BASES]
WSTARTS = np.concatenate([[0], np.cumsum(WSIZES)]).astype(np.int64)
NSTAGE = int(WSTARTS[-1]) + 1  # +1 pad row for the last pair descriptor
PIECE = int(os.environ.get("KERNEL_PIECE", "8"))  # pair-columns per
# dma_gather (1024 idx = the gather ucode idx ring limit); at the
# current DMA load the drain no longer blocks the descriptor ring

LAST_EXEC_NS = None


def _ensure_ntff_hook():
    """Shim antenv.axon_hooks if the image's antenv predates it."""
    import types

    try:
        from antenv.axon_hooks import get_axon_ntff_profile_hook  # noqa: F401

        return
    except ImportError:
        pass
    try:
        import antenv

        mod = types.ModuleType("antenv.axon_hooks")
        _hook = [None]
        mod.set_axon_ntff_profile_hook = lambda h: _hook.__setitem__(0, h)
        mod.get_axon_ntff_profile_hook = lambda: _hook[0]
        antenv.axon_hooks = mod
        sys.modules["antenv.axon_hooks"] = mod
        from trn_agent_boot.trn_boot import _ntff_profile_via_ctypes

        mod.set_axon_ntff_profile_hook(
            _ntff_profile_via_ctypes("/opt/axon/libaxon_pjrt.so")
        )
    except Exception:
        pass


def _build_program(sched) -> bass.Bass:
    """One SPMD program; every core runs it on its own edge shard."""
    nc = bacc.Bacc(num_swdge_queues=4)
    f32 = mybir.dt.float32
    bf16 = mybir.dt.bfloat16
    i16 = mybir.dt.int16

    caps_p = sched["caps_p"]  # [N_BLOCKS, SEG] pair-cols per (block, window)
    acts = sched["acts"]  # per block: list of active (window, paircol, half)
    w_b = np.asarray([len(a) for a in acts])  # matmuls per block
    maxw = int(w_b.max())
    col_off = np.concatenate([[0], np.cumsum(w_b)])
    ncols = int(col_off[-1])
    # pair-col index of (b, s, 0) within window s's gather stream
    prefix_p = np.concatenate(
        [np.zeros((1, SEG), dtype=np.int64), np.cumsum(caps_p, axis=0)], axis=0
    )
    n_pcols = prefix_p[-1]  # [SEG]
    n_pieces = [(int(nq) + PIECE - 1) // PIECE for nq in n_pcols]
    ipp = PIECE * P // 16  # idx cols per piece (64)
    idx_off = np.concatenate([[0], np.cumsum([nq * ipp for nq in n_pieces])])
    idxcols = int(idx_off[-1])

    w_d = nc.declare_dram_parameter("w", [NSTAGE, FEAT], bf16, isOutput=False)
    gidx_d = nc.declare_dram_parameter("gidx", [P, idxcols], i16, isOutput=False)
    dstloc_d = nc.declare_dram_parameter("dstloc", [P, ncols], bf16, isOutput=False)
    cib_d = nc.declare_dram_parameter("cib", [P, N_BLOCKS], f32, isOutput=False)
    iota_d = nc.declare_dram_parameter("iota", [P, maxw * P], bf16, isOutput=False)
    h_d = nc.declare_dram_parameter("h", [DST_PAD, FEAT], f32, isOutput=True)

    with tile.TileContext(nc) as tc:
        with (
            tc.tile_pool(name="meta", bufs=1) as meta,
            tc.tile_pool(name="gather", bufs=6) as gpool,
            tc.tile_pool(name="work", bufs=3) as work,
            tc.tile_pool(name="out", bufs=3) as opool,
            tc.tile_pool(name="psum", bufs=4, space="PSUM") as psum,
        ):
            gidx = meta.tile([P, idxcols], i16)
            dstloc = meta.tile([P, ncols], bf16)
            cib = meta.tile([P, N_BLOCKS], f32)
            # head pieces first so the first gathers start early
            for s in range(SEG):
                lo = int(idx_off[s])
                mid = min(lo + 2 * ipp, int(idx_off[s + 1]))
                nc.sync.dma_start(out=gidx[:, lo:mid], in_=gidx_d[:, lo:mid])
            nc.sync.dma_start(out=dstloc[:], in_=dstloc_d[:])
            for s in range(SEG):
                mid = min(int(idx_off[s]) + 2 * ipp, int(idx_off[s + 1]))
                hi = int(idx_off[s + 1])
                if hi > mid:
                    nc.sync.dma_start(out=gidx[:, mid:hi], in_=gidx_d[:, mid:hi])
            nc.sync.dma_start(out=cib[:], in_=cib_d[:])

            # iota[p, c*128 + j] = j  (dst slot within block), host-built
            iota = meta.tile([P, maxw * P], bf16)
            nc.sync.dma_start(out=iota[:], in_=iota_d[:])

            # issue all paired gathers; Tile paces them via pool bufs.
            # One 512B descriptor per pair-slot: rows B[t], B[t+1].
            gts: list[dict] = [{} for _ in range(SEG)]
            for pc in range(max(n_pieces)):
                for s in range(SEG):
                    if pc >= n_pieces[s]:
                        continue
                    npair = min(PIECE, int(n_pcols[s]) - pc * PIECE)
                    gt = gpool.tile([P, PIECE * 2 * FEAT], bf16, tag=f"gw{s}")
                    in_ap = bass.AP(
                        w_d[:, :].tensor,
                        int(WSTARTS[s]) * FEAT,
                        [(FEAT, WSIZES[s]), (1, 2 * FEAT)],
                    )
                    co = int(idx_off[s]) + pc * ipp
                    nc.gpsimd.dma_gather(
                        gt[:, : npair * 2 * FEAT].rearrange(
                            "p (m f) -> p m f", f=2 * FEAT
                        ),
                        in_ap,
                        gidx[:, co : co + npair * P // 16],
                        npair * P,
                        npair * P,
                        2 * FEAT,
                        elem_step=FEAT,
                        queue_num=s,
                    )
                    gts[s][pc] = gt

            for b in range(N_BLOCKS):
                wb = int(w_b[b])
                co = int(col_off[b])
                onehot = work.tile([P, maxw * P], bf16, tag="onehot")
                nc.vector.tensor_tensor(
                    out=onehot[:, : wb * P].rearrange("p (m f) -> p m f", f=P),
                    in0=dstloc[:, co : co + wb].to_broadcast([P, wb, P]),
                    in1=iota[:, : wb * P].rearrange("p (m f) -> p m f", f=P),
                    op=mybir.AluOpType.is_equal,
                )
                acc = psum.tile([P, FEAT], f32, tag="acc")
                for j, (s, jj, half) in enumerate(acts[b]):
                    jp = int(prefix_p[b, s]) + jj  # global pair-col
                    gt = gts[s][jp // PIECE]
                    off = jp % PIECE
                    nc.tensor.matmul(
                        out=acc[:],
                        lhsT=onehot[:, j * P : (j + 1) * P],
                        rhs=gt[
                            :,
                            off * 2 * FEAT + half * FEAT : off * 2 * FEAT
                            + (half + 1) * FEAT,
                        ],
                        start=(j == 0),
                        stop=(j == wb - 1),
                    )
                ho = opool.tile([P, FEAT], f32, tag="ho")
                nc.scalar.mul(ho[:], acc[:], cib[:, b : b + 1])
                nc.sync.dma_start(out=h_d[b * P : (b + 1) * P, :], in_=ho[:])
    return nc


class _DSU:
    __slots__ = ("p",)

    def __init__(self, n):
        self.p = list(range(n))

    def find(self, x):
        p = self.p
        while p[x] != x:
            p[x] = p[p[x]]
            x = p[x]
        return x

    def union(self, a, b):
        self.p[self.find(a)] = self.find(b)


def _prep_inputs(weight, cj, ci, src, dst):
    """Partition edges by dst owner; pair edges; build metadata."""
    import ml_dtypes

    order = np.argsort(dst, kind="stable")
    ds = dst[order].astype(np.int64)
    ss = src[order].astype(np.int64)
    core_bounds = np.searchsorted(ds, np.arange(N_CORES + 1) * DST_PER_CORE)

    percore = []
    perms = []
    for c in range(N_CORES):
        a, b = core_bounds[c], core_bounds[c + 1]
        d_local = ds[a:b] - c * DST_PER_CORE
        g = ss[a:b]

        # dst->block packing: heaviest 128 dsts to the overflow block,
        # snake the rest so all other blocks carry <= 2048 edges.
        deg = np.bincount(d_local, minlength=DST_PER_CORE)
        order_d = np.argsort(-deg, kind="stable")
        blk_of = np.empty(DST_PER_CORE, dtype=np.int64)
        slot_of = np.empty(DST_PER_CORE, dtype=np.int64)
        hot = order_d[:P]
        blk_of[hot] = N_BLOCKS - 1
        slot_of[hot] = np.arange(P)
        rest = order_d[P:]
        nb = N_BLOCKS - 1
        for i in range(0, len(rest), nb):
            seg_d = rest[i : i + nb]
            row = i // nb
            blks = np.arange(len(seg_d))
            if row % 2:
                blks = nb - 1 - blks
            blk_of[seg_d] = blks
            slot_of[seg_d] = row
        perms.append((blk_of, slot_of))

        block = blk_of[d_local]
        o2 = np.lexsort((g, block))
        d_local, g, block = d_local[o2], g[o2], block[o2]
        bb = np.searchsorted(block, np.arange(N_BLOCKS + 1))
        percore.append((d_local, g, bb))

    # --- pairing + window fill -------------------------------------------
    # caps_p in pair-columns; windows 0-2 start at 2 and bump on overflow
    caps_p = np.full((N_BLOCKS, SEG), 2, dtype=np.int64)
    for attempt in range(6):
        overflow = np.zeros((N_BLOCKS, 3), dtype=bool)
        results = []  # per core: (pairs, halves) per (block, window)
        load3 = np.zeros((N_CORES, N_BLOCKS), dtype=np.int64)
        for c in range(N_CORES):
            d_local, g, bb = percore[c]
            # per-window pairing state over VIRTUAL row ids: originals
            # 0..WSIZE-1 plus up to BUD duplicated copies (a copy of a
            # row gets fresh degree-2 capacity in the B_w ordering)
            BUD = int(os.environ.get("KERNEL_BUD", "8000"))
            degv = [
                np.zeros(WSIZES[s] + BUD + 1, dtype=np.int8) for s in range(SEG)
            ]
            dsu = [_DSU(WSIZES[s] + BUD + 1) for s in range(SEG)]
            curv = [
                np.full(WSIZES[s], -1, dtype=np.int64) for s in range(SEG)
            ]
            copy_origs = [[] for _ in range(SEG)]
            placed = np.zeros(len(g), dtype=bool)
            core_res = [[None] * SEG for _ in range(N_BLOCKS)]
            for s in range(SEG):
                lo_v, hi_v = BASES[s], BASES[s] + WIN
                nxt = BASES[s + 1] if s < 3 else N_NODES
                norig = WSIZES[s]
                D = degv[s]
                U = dsu[s]
                cur = curv[s]
                cpo = copy_origs[s]
                for blk in range(N_BLOCKS):
                    i0, i1 = bb[blk], bb[blk + 1]
                    idxs = np.arange(i0, i1)[~placed[i0:i1]]
                    vals = g[idxs]
                    idxs = idxs[(vals >= lo_v) & (vals < hi_v)]
                    cap_slots = int(caps_p[blk, s]) * P
                    slots = []  # [e1, e2, vid1, vid2]; one descriptor each
                    open_h = []  # indices of slots missing a second half
                    for e in idxs:
                        u0 = int(g[e]) - lo_v
                        u = cur[u0] if cur[u0] >= 0 else u0
                        if D[u] >= 2 and len(cpo) < BUD:
                            u = norig + len(cpo)
                            cpo.append(u0)
                            cur[u0] = u
                        done = False
                        if D[u] < 2:
                            for t in range(len(open_h) - 1, -1, -1):
                                se = slots[open_h[t]]
                                v = se[2]
                                if D[v] >= 2:
                                    # re-key the stale half to a fresh copy
                                    if len(cpo) < BUD:
                                        v0 = v if v < norig else cpo[v - norig]
                                        v = norig + len(cpo)
                                        cpo.append(v0)
                                        cur[v0] = v
                                        se[2] = v
                                    else:
                                        open_h.pop(t)
                                        continue
                                if v == u or U.find(u) == U.find(v):
                                    if len(open_h) - t >= 16:
                                        break
                                    continue
                                se[1] = e
                                se[3] = u
                                D[u] += 1
                                D[v] += 1
                                U.union(u, v)
                                open_h.pop(t)
                                placed[e] = True
                                done = True
                                break
                        if done:
                            continue
                        if s == 3 or len(slots) < cap_slots:
                            open_h.append(len(slots))
                            slots.append([e, -1, u, -1])
                            placed[e] = True
                        elif int(g[e]) < nxt:
                            # a must-edge that neither fit nor paired
                            overflow[blk, s] = True
                        # else: eligible for the next window; leave it
                    pairs = [tuple(sl) for sl in slots if sl[1] >= 0]
                    halves = [(sl[0], sl[2]) for sl in slots if sl[1] < 0]
                    core_res[blk][s] = (pairs, halves)
                    if s == 3:
                        load3[c, blk] = len(slots)
            if not overflow.any():
                assert placed.all(), f"core {c}: {int((~placed).sum())} edges lost"
            results.append((core_res, copy_origs))
        if not overflow.any():
            break
        for blk in range(N_BLOCKS):
            for s in range(3):
                if overflow[blk, s]:
                    caps_p[blk, s] += 1
    caps_p[:, 3] = np.maximum(1, -(-load3.max(axis=0) // P))

    # active chunk columns: second halves of a pair-col carry edges only
    # where some core placed a pair there (union over cores keeps the
    # layout SPMD-uniform); dead columns get no one-hot and no matmul
    maxpb = np.zeros((N_BLOCKS, SEG), dtype=np.int64)
    maxsl = np.zeros((N_BLOCKS, SEG), dtype=np.int64)
    for c in range(N_CORES):
        for blk in range(N_BLOCKS):
            for s in range(SEG):
                pairs, halves = results[c][0][blk][s]
                maxpb[blk, s] = max(maxpb[blk, s], len(pairs))
                maxsl[blk, s] = max(maxsl[blk, s], len(pairs) + len(halves))
    acts = []
    colmap = {}
    col_off = [0]
    for blk in range(N_BLOCKS):
        al = []
        for s in range(SEG):
            for jj in range(int(caps_p[blk, s])):
                if maxsl[blk, s] > jj * P:
                    colmap[(blk, s, jj, 0)] = col_off[-1] + len(al)
                    al.append((s, jj, 0))
                if maxpb[blk, s] > jj * P:
                    colmap[(blk, s, jj, 1)] = col_off[-1] + len(al)
                    al.append((s, jj, 1))
        acts.append(al)
        col_off.append(col_off[-1] + len(al))
    col_off = np.asarray(col_off)
    ncols = int(col_off[-1])
    w_b = np.diff(col_off)
    prefix_p = np.concatenate(
        [np.zeros((1, SEG), dtype=np.int64), np.cumsum(caps_p, axis=0)], axis=0
    )
    n_pcols = prefix_p[-1]
    n_pieces = [(int(nq) + PIECE - 1) // PIECE for nq in n_pcols]
    ipp = PIECE * P // 16
    idx_off = np.concatenate([[0], np.cumsum([nq * ipp for nq in n_pieces])])
    idxcols = int(idx_off[-1])

    sched = {"caps_p": caps_p, "acts": acts}
    maxw = int(w_b.max())
    iota_arr = np.tile(np.arange(P, dtype=np.float32), (P, maxw)).astype(
        ml_dtypes.bfloat16
    )

    cj_flat = cj.reshape(-1).astype(np.float32)
    ci_flat = ci.reshape(-1).astype(np.float32)
    wc = (weight * cj_flat[:, None]).astype(ml_dtypes.bfloat16)

    in_maps = []
    npairs_tot = 0
    for c in range(N_CORES):
        blk_of, slot_of = perms[c]
        d_local, g, bb = percore[c]
        core_res, copy_origs = results[c]

        # B_w orderings from the pairing adjacencies (linear forest over
        # virtual row ids: originals then duplicated copies)
        posB = []
        stage_rows = np.empty(NSTAGE, dtype=np.int64)
        for s in range(SEG):
            nw = WSIZES[s]
            cpo = copy_origs[s]
            nv = nw + len(cpo)
            orig_of = np.concatenate(
                [np.arange(nw, dtype=np.int64), np.asarray(cpo, dtype=np.int64)]
            )
            A = {}
            refd = set()
            for blk in range(N_BLOCKS):
                pairs, halves = core_res[blk][s]
                for _, _, v1, v2 in pairs:
                    A.setdefault(v1, []).append(v2)
                    A.setdefault(v2, []).append(v1)
                    refd.add(v1)
                    refd.add(v2)
                for _, vv in halves:
                    refd.add(vv)
            pos = np.full(nv + 1, -1, dtype=np.int64)
            cur = 0
            visited = np.zeros(nv, dtype=bool)
            stage_win = np.zeros(nw, dtype=np.int64)  # default row 0 of window
            # path endpoints first (degree 1); cycles are prevented by the
            # DSU, so every component is a path
            for start in A:
                if visited[start] or len(A[start]) != 1:
                    continue
                node, prev = start, -1
                while True:
                    pos[node] = cur
                    stage_win[cur] = orig_of[node]
                    cur += 1
                    visited[node] = True
                    nxt_n = -1
                    for cand in A[node]:
                        if cand != prev and not visited[cand]:
                            nxt_n = cand
                            break
                    if nxt_n < 0:
                        break
                    prev, node = node, nxt_n
            for vv in refd:
                if pos[vv] < 0:
                    pos[vv] = cur
                    stage_win[cur] = orig_of[vv]
                    cur += 1
            assert cur <= nw, f"window {s}: {cur} > {nw} B positions"
            posB.append(pos)
            stage_rows[WSTARTS[s] : WSTARTS[s + 1]] = stage_win + BASES[s]
        stage_rows[-1] = 0
        wstage = wc[stage_rows]

        dstloc = np.full((P, ncols), -1, dtype=ml_dtypes.bfloat16)
        pairidx = np.zeros((P, int(n_pcols.sum())), dtype=np.int16)
        pcol_off = np.concatenate([[0], np.cumsum(n_pcols)])
        for s in range(SEG):
            pos = posB[s]
            qbase = int(pcol_off[s])
            for blk in range(N_BLOCKS):
                pairs, halves = core_res[blk][s]
                npairs_tot += len(pairs)
                q0 = qbase + int(prefix_p[blk, s])
                i = 0
                for e1, e2, v1, v2 in pairs:
                    u = pos[v1]
                    v = pos[v2]
                    jj, pp = i // P, i % P
                    if u + 1 == v:
                        t, h0, h1 = u, e1, e2
                    else:
                        assert v + 1 == u, (u, v)
                        t, h0, h1 = v, e2, e1
                    pairidx[pp, q0 + jj] = t
                    dstloc[pp, colmap[(blk, s, jj, 0)]] = slot_of[
                        d_local[h0]
                    ].astype(ml_dtypes.bfloat16)
                    dstloc[pp, colmap[(blk, s, jj, 1)]] = slot_of[
                        d_local[h1]
                    ].astype(ml_dtypes.bfloat16)
                    i += 1
                for e, vv in halves:
                    t = pos[vv]
                    jj, pp = i // P, i % P
                    pairidx[pp, q0 + jj] = t
                    dstloc[pp, colmap[(blk, s, jj, 0)]] = slot_of[
                        d_local[e]
                    ].astype(ml_dtypes.bfloat16)
                    i += 1

        # pack pair indices: per (window, piece), idx j at
        # [16*rep + j%16, j//16]; j = (paircol_within_piece*128 + p)
        gidx = np.zeros((P, idxcols), dtype=np.int16)
        for s in range(SEG):
            vals = pairidx[
                :, int(pcol_off[s]) : int(pcol_off[s + 1])
            ].T.reshape(-1)
            vals = np.pad(vals, (0, n_pieces[s] * PIECE * P - len(vals)))
            block16 = vals.reshape(n_pieces[s] * ipp, 16).T
            gidx[:, int(idx_off[s]) : int(idx_off[s + 1])] = np.tile(
                block16, (8, 1)
            )

        ci_core = ci_flat[c * DST_PER_CORE : (c + 1) * DST_PER_CORE]
        cib_arr = np.zeros((N_BLOCKS, P), dtype=np.float32)
        cib_arr[blk_of, slot_of] = ci_core
        cib = cib_arr.T.copy()

        in_maps.append(
            {
                "w": wstage,
                "gidx": gidx,
                "dstloc": dstloc,
                "cib": cib,
                "iota": iota_arr,
            }
        )
    if int(os.environ.get("KERNEL_STATS", "0")):
        tot_slots = int(n_pcols.sum()) * P * N_CORES
        print(
            f"[prep] pair-cols/core={int(n_pcols.sum())} slots/core="
            f"{int(n_pcols.sum()) * P} pairs(all cores)={npairs_tot} "
            f"edges={len(ds)} fill={(len(ds) + npairs_tot) / tot_slots:.3f}"
        )
    return in_maps, sched, perms


def _maybe_enable_ldw_opt():
    if not int(os.environ.get("KERNEL_LDW", "0")):
        return
    import concourse.bass_utils as _bu

    if getattr(_bu, "_ldw_patched", False):
        return
    _orig = _bu.run_command

    def _patched(argv, **kw):
        argv = [
            "--enable-ldw-opt=true" if a == "--enable-ldw-opt=false" else a
            for a in argv
        ]
        return _orig(argv, **kw)

    _bu.run_command = _patched
    _bu._ldw_patched = True


def kernel(weight, cj, ci, src, dst):
    global LAST_EXEC_NS
    _maybe_enable_ldw_opt()
    weight = np.asarray(weight, dtype=np.float32)
    cj = np.asarray(cj, dtype=np.float32)
    ci = np.asarray(ci, dtype=np.float32)
    src = np.asarray(src, dtype=np.int32)
    dst = np.asarray(dst, dtype=np.int32)

    in_maps, sched, perms = _prep_inputs(weight, cj, ci, src, dst)
    nc = _build_program(sched)
    nc.finalize()
    trace = bool(int(os.environ.get("KERNEL_TRACE", "0")))
    if trace:
        _ensure_ntff_hook()
    try:
        res = run_bass_kernel_spmd(
            nc, in_maps, core_ids=list(range(N_CORES)), trace=trace
        )
    except Exception:
        # one retry without tracing — transient device errors
        # (NRT_EXEC_UNIT_UNRECOVERABLE) usually clear on the next attempt
        res = run_bass_kernel_spmd(
            nc, in_maps, core_ids=list(range(N_CORES)), trace=False
        )
    LAST_EXEC_NS = res.exec_time_ns
    parts = []
    for c in range(N_CORES):
        blk_of, slot_of = perms[c]
        h = res.results[c]["h"]
        parts.append(h[blk_of * P + slot_of])
    return np.concatenate(parts, axis=0).astype(np.float32)
